# revision 1
# baseline (speedup 1.0000x reference)
"""Trainium2 Bass kernel for nn_DeConvAfterDownSampling.

Math (from the reference): with s[n] = sum_w x[b,c,h,w] flattened over
n = (b,c,h), Wf = W.reshape(F, P):

    out[0, f, n, p] = relu(s[n] * Wf[f, p] + b[f])      # (1, F, N, P)

N = 8*64*64 = 32768, F = 64, P = 25.  Output is ~210 MB fp32 while inputs
are ~8 MB, so the kernel is bound by the output HBM write.

Sharding: data-parallel over n across 8 cores (N_LOCAL = 4096 per core);
W and b replicated; no cross-core communication.

Per-core plan (partitions = (h, f) with h in {0,1} stacking two n-halves
so all 128 partitions are used; n is tiled as [256, 256, 512 x 7] — small
first tiles shorten the pipeline-fill ramp):
  1. Per-tile DMAs load x chunks in natural row order (partition q holds
     rpp = tn/128 consecutive rows -> rpp*256 B contiguous per partition,
     full DMA rate).
  2. PE transposes each (128 q, 64 w) row-group c -> (64 w, 128 j) in
     PSUM (column j <-> n = n0 + j*rpp + c); VectorE copies split the
     columns into partition halves, assembling xT (128=(h,w), rpp, 64).
  3. One K=128 matmul with a constant block-diagonal ones matrix E
     (E[(h',w),(h,f)] = (h==h')) reduces over w AND broadcasts across all
     f at once; its rhs AP streams (j', c) so the PSUM free axis comes
     out in exact n order: s_bcast[(h,f), m] = s[n0 + (tn/2)h + m].
  4. For each p in 0..24 one elementwise op computes
     relu(W[f,p] * s + b[f]) with W[:,p] as per-partition scale (and, when
     b != 0, b as per-partition bias), writing the (stride 25) p-slice of
     the output tile.  Ops are split across ScalarE (activation, reads
     s from PSUM), GpSimd and VectorE (tensor_scalar mult + max 0, read
     an SBUF copy of s).
  5. One 1.6-3.3 MB DMA per tile writes the (128, tn/2, 25) tile to HBM;
     the per-partition free layout (n-major, p-minor) is exactly
     contiguous HBM order, so each partition is one contiguous chunk.

TimelineSim cost model: 82.6 us per core (output-DMA bound: 26.2 MB fp32
written per core at ~360 GB/s = 72.8 us with zero inter-DMA gaps, 8.4 us
pipeline-fill ramp, 1.4 us drain).  Rel err vs the jax reference: 3.3e-07.
"""

import numpy as np

import concourse.bass as bass
import concourse.mybir as mybir
from concourse import bacc, masks, tile
from concourse.bass_utils import run_bass_kernel_spmd
from concourse.tile_rust import add_dep_helper

F32 = mybir.dt.float32

N_CORES = 8
B, C, H, WDIM = 8, 64, 64, 64
F, P = 64, 25
N_TOTAL = B * C * H          # 32768
N_LOCAL = N_TOTAL // N_CORES  # 4096
# Per-tile n sizes: small first tiles shorten the pipeline-fill ramp;
# large tail tiles make the output DMAs 6.5 MB for real-HW DMA efficiency
# (descriptor overhead amortizes with transfer size).
TILE_SIZES = [256, 256, 512, 512, 512, 1024, 1024]
assert sum(TILE_SIZES) == N_LOCAL
NPART = 128

# Engine split for the 25 per-p elementwise ops (b == 0 fast path):
# ScalarE activation / GpSimd tensor_scalar / VectorE tensor_scalar.
# Small (ramp) tiles bias away from ScalarE, whose sequencer is slow.
def engine_split(tn):
    if tn <= 128:
        return set(range(3)), set(range(3, 10))
    if tn <= 256:
        return set(range(4)), set(range(4, 11))
    if tn == 512:
        return set(range(6)), set(range(6, 13))
    return set(range(8)), set(range(8, 14))


def build_bass(with_bias: bool) -> bass.Bass:
    nc = bacc.Bacc(None)

    x_d = nc.dram_tensor("x", (N_LOCAL, WDIM), F32, kind="ExternalInput")
    w_d = nc.dram_tensor("W", (F, P), F32, kind="ExternalInput")
    b_d = nc.dram_tensor("b", (F, 1), F32, kind="ExternalInput")
    o_d = nc.dram_tensor("out", (F, N_LOCAL, P), F32, kind="ExternalOutput")

    with tile.TileContext(nc) as tc:
        with (
            tc.tile_pool(name="const", bufs=1) as constp,
            tc.tile_pool(name="xin", bufs=1) as xinp,
            tc.tile_pool(name="work", bufs=3) as workp,
            tc.tile_pool(name="outp", bufs=3) as outp,
            tc.tile_pool(name="psum", bufs=2, space="PSUM") as psump,
            tc.tile_pool(name="psum3", bufs=3, space="PSUM") as psump3,
        ):
            # Pool builds identity + E first: they gate the PE transposes.
            ident = constp.tile([NPART, NPART], F32)
            masks.make_identity(nc, ident[:])

            # Block-diagonal ones: E[k, i] = 1 iff k//64 == i//64.
            e_mat = constp.tile([NPART, NPART], F32)
            nc.gpsimd.memset(e_mat[:], 0.0)
            nc.gpsimd.memset(e_mat[0:64, 0:64], 1.0)
            nc.gpsimd.memset(e_mat[64:128, 64:128], 1.0)

            # Pull the ACT table load (~1.3 us) off the critical path: a
            # dummy Relu at t=0 makes insert_act_table_loads put it first.
            warm = constp.tile([NPART, 1], F32)
            nc.vector.memset(warm[:], 0.0)
            warm_out = constp.tile([NPART, 1], F32)
            nc.scalar.activation(
                warm_out[:], warm[:], mybir.ActivationFunctionType.Relu
            )

            # W columns replicated on both partition halves, via the ACT
            # HWDGE ring so neither the x loads (SP ring) nor the Pool
            # engine (identity/E) are delayed.
            wcols = constp.tile([NPART, P], F32)
            nc.scalar.dma_start(wcols[0:64, :], w_d[:, :])
            nc.scalar.dma_start(wcols[64:128, :], w_d[:, :])
            if with_bias:
                bcol = constp.tile([NPART, 1], F32)
                nc.scalar.dma_start(bcol[0:64, :], b_d[:, :])
                nc.scalar.dma_start(bcol[64:128, :], b_d[:, :])
                bias_arg = bcol[:, 0:1]
            else:
                # b is all zeros: skip the load, use an immediate bias.
                nc.gpsimd.dma_start(constp.tile([1, 1], F32, name="bjunk")[:],
                                    b_d[0:1, :])  # keep "b" a live input
                bias_arg = 0.0

            tile_offsets = [sum(TILE_SIZES[:u]) for u in range(len(TILE_SIZES))]

            # --- load x (critical path), one chunk per tile ---
            # Natural row order: partition q holds rpp = tn/128 consecutive
            # rows (rpp*256 B contiguous per partition -> full DMA rate).
            # n = n0 + q*rpp + c.
            x_chunks = []
            for u, (n0, tn) in enumerate(zip(tile_offsets, TILE_SIZES)):
                rpp = tn // NPART
                x_ch = xinp.tile(
                    [NPART, rpp, WDIM], F32, name=f"xch{u}", tag=f"xch{u}"
                )
                nc.sync.dma_start(
                    x_ch[:],
                    x_d[n0 : n0 + tn, :].rearrange("(q c) w -> q c w", c=rpp),
                )
                x_chunks.append(x_ch)

            # Per-engine chaining of the elementwise ops in program order so
            # the scheduler finishes tile u before starting tile u+1 ops —
            # otherwise cross-tile interleaving delays the first out DMA.
            prev_op = {}

            def chain(key, bi):
                if key in prev_op:
                    add_dep_helper(
                        bi.ins, prev_op[key].ins, sync=False, reason="tile op order"
                    )
                prev_op[key] = bi

            for u, (n0, tn) in enumerate(zip(tile_offsets, TILE_SIZES)):
                rpp = tn // NPART
                half = tn // 2
                out_r = o_d[:, n0 : n0 + tn, :].rearrange(
                    "f (h j) p -> h f j p", h=2, j=half
                )  # (2, 64, half, P)

                # --- transpose row-groups: xT[w, c, j] = x[n0 + j*rpp + c, w]
                xt_ps = psump.tile([64, rpp, NPART], F32, name="xtp", tag="xtp")
                for c in range(rpp):
                    chain(
                        "pe",
                        nc.tensor.transpose(
                            xt_ps[:, c, :], x_chunks[u][:, c, :], ident[:]
                        ),
                    )
                # split transpose columns j = 64h + j' into partition halves
                xt_sb = workp.tile([NPART, rpp, 64], F32, tag="xt_sb")
                chain("v", nc.vector.tensor_copy(xt_sb[0:64], xt_ps[:, :, 0:64]))
                chain("v", nc.vector.tensor_copy(xt_sb[64:128], xt_ps[:, :, 64:128]))

                # --- s broadcast: one matmul, K=128; rhs streams (j', c) so
                # the free axis is n-order: m = j'*rpp + c ---
                s_ps = psump3.tile([NPART, half], F32, tag="s_ps")
                chain(
                    "pe",
                    nc.tensor.matmul(
                        s_ps[:], e_mat[:], xt_sb[:].rearrange("k c j -> k j c")
                    ),
                )

                s_sb = workp.tile([NPART, half], F32, tag="s_sb")
                chain("v", nc.vector.tensor_copy(s_sb[:], s_ps[:]))

                # --- 25 per-p elementwise ops ---
                out_t = outp.tile([NPART, half, P], F32, tag="out_t")
                scalar_ps, gpsimd_ps = engine_split(tn)
                for p in range(P):
                    if with_bias or p in scalar_ps:
                        bi = nc.scalar.activation(
                            out_t[:, :, p],
                            s_ps[:],
                            mybir.ActivationFunctionType.Relu,
                            bias=bias_arg,
                            scale=wcols[:, p : p + 1],
                        )
                        chain("s", bi)
                    else:
                        gp = p in gpsimd_ps
                        eng = nc.gpsimd if gp else nc.vector
                        bi = eng.tensor_scalar(
                            out_t[:, :, p],
                            s_sb[:],
                            wcols[:, p : p + 1],
                            0.0,
                            mybir.AluOpType.mult,
                            mybir.AluOpType.max,
                        )
                        chain("g" if gp else "v", bi)

                nc.sync.dma_start(out_r, out_t[:])

    nc.compile()
    return nc


_CACHE: dict[bool, bass.Bass] = {}


def _get_bass(with_bias: bool) -> bass.Bass:
    if with_bias not in _CACHE:
        _CACHE[with_bias] = build_bass(with_bias)
    return _CACHE[with_bias]


last_exec_time_ns = None
last_profile = None


def kernel(x, W, b, trace=False, **run_kwargs):
    global last_exec_time_ns, last_profile
    x = np.ascontiguousarray(np.asarray(x, dtype=np.float32)).reshape(N_TOTAL, WDIM)
    wf = np.ascontiguousarray(np.asarray(W, dtype=np.float32)).reshape(F, P)
    bf = np.ascontiguousarray(np.asarray(b, dtype=np.float32)).reshape(F, 1)

    nc = _get_bass(bool(np.any(bf)))

    in_maps = [
        {
            "x": x[m * N_LOCAL : (m + 1) * N_LOCAL],
            "W": wf,
            "b": bf,
        }
        for m in range(N_CORES)
    ]
    res = run_bass_kernel_spmd(
        nc, in_maps, core_ids=list(range(N_CORES)), trace=trace, **run_kwargs
    )
    last_exec_time_ns = res.exec_time_ns
    last_profile = res.profile_json
    outs = [np.asarray(res.results[m]["out"]) for m in range(N_CORES)]
    full = np.concatenate(outs, axis=1)  # (F, N_TOTAL, P)
    return full[None]

